# revision 5
# baseline (speedup 1.0000x reference)
"""Trainium2 Bass kernel for DiagonalLinear: y = x * diag (elementwise scale of last axis).

x: [4, 4096, 4096] f32, diag: [4096] f32 -> y: [4, 4096, 4096] f32.

Data-parallel over 8 NeuronCores. The two NeuronCores of an HBM stack see
asymmetric DMA bandwidth under mutual contention (odd cores sustain the
~425 GB/s fabric ceiling, even cores ~355 GB/s), so the row split within
each pair is rebalanced 2176/1920 instead of 2048/2048. All cores run the
same SPMD program with 17 row-tiles; on even cores the last 2 tiles' DMAs
are predicated off via `cond=` (semaphores still fire, so the pipeline is
unchanged).

Per tile: DMA [128, 4096] f32 (2 MiB) into SBUF, DVE multiply by a diag
tile replicated across partitions (loaded once, 16 KiB, broadcast
on-chip), DMA back out. Memory-bound; DVE hides under the DMA stream.
"""

import numpy as np

import concourse.bass as bass
import concourse.bacc as bacc
import concourse.mybir as mybir
import concourse.tile as tile
from concourse.bass_utils import run_bass_kernel_spmd

BATCH, SEQ, SIZE = 4, 4096, 4096
N_CORES = 8
ROWS = BATCH * SEQ                   # 16384
PAIR_ROWS = ROWS // (N_CORES // 2)   # 4096 rows per HBM-stack pair
P = 128                              # SBUF partitions
NT = 17                              # tiles in the program (= odd-core tiles)
NT_EVEN = 15                         # tiles actually transferred on even cores
ROWS_ODD = NT * P                    # 2176
ROWS_EVEN = NT_EVEN * P              # 1920
FP32 = mybir.dt.float32

_built = None


def _build():
    """Build + schedule the shared SPMD Tile kernel."""
    nc = bacc.Bacc("TRN2", target_bir_lowering=False, debug=False)

    x = nc.dram_tensor("x", [NT, P, SIZE], FP32, kind="ExternalInput").ap()
    d = nc.dram_tensor("diag", [SIZE], FP32, kind="ExternalInput").ap()
    flag = nc.dram_tensor("flag", [1, 1], mybir.dt.uint32, kind="ExternalInput").ap()
    y = nc.dram_tensor("y", [NT, P, SIZE], FP32, kind="ExternalOutput").ap()

    with tile.TileContext(nc) as tc:
        with (
            tc.tile_pool(name="dpool", bufs=1) as dpool,
            tc.tile_pool(name="xpool", bufs=8) as xpool,
        ):
            # Queue the first x loads before anything that stalls the SP
            # sequencer (the value_load below waits ~2 us on its DMA
            # completion; with these loads already in the ring the DMA
            # engines stay busy through that stall).
            HEAD = 6
            head_tiles = []
            for t in range(HEAD):
                xt = xpool.tile([P, SIZE], FP32)
                nc.sync.dma_start(out=xt[:], in_=x[t])
                head_tiles.append(xt)

            # Load diag (16 KiB) into partition 0, replicate across all
            # 128 partitions on-chip — no bulk HBM traffic for the broadcast.
            d0 = dpool.tile([1, SIZE], FP32)
            nc.sync.dma_start(out=d0[:], in_=d[None, :])
            dtile = dpool.tile([P, SIZE], FP32)
            nc.gpsimd.partition_broadcast(dtile[:], d0[:])

            # Per-core tile-count flag (1 = run all NT tiles, 0 = skip the
            # last NT - NT_EVEN). One register per issuing engine; the cond
            # must be a bool expression so ap_or_oob's [0,1] range assert
            # folds away (a SeqAssert instruction breaks this runtime).
            f_sb = dpool.tile([1, 1], mybir.dt.uint32)
            nc.sync.dma_start(out=f_sb[:], in_=flag[:])
            cond_ld = nc.sync.value_load(f_sb[:1, :1]) >= 1
            cond_st = nc.scalar.value_load(f_sb[:1, :1]) >= 1

            for t in range(NT):
                if t < HEAD:
                    xt = head_tiles[t]
                else:
                    xt = xpool.tile([P, SIZE], FP32)
                    if t < NT_EVEN:
                        nc.sync.dma_start(out=xt[:], in_=x[t])
                    else:
                        # Skipped on even cores; memset keeps the tile
                        # defined so the (dead) multiply reads initialized
                        # memory.
                        nc.vector.memset(xt[:], 0.0)
                        nc.sync.dma_start(out=xt[:], in_=x[t], cond=cond_ld)
                nc.vector.tensor_mul(xt[:], xt[:], dtile[:])
                if t < NT_EVEN:
                    nc.scalar.dma_start(out=y[t], in_=xt[:])
                else:
                    nc.scalar.dma_start(out=y[t], in_=xt[:], cond=cond_st)

    nc.compile()
    return nc


def _get_nc():
    global _built
    if _built is None:
        _built = _build()
    return _built


def _make_in_maps(x: np.ndarray, diag: np.ndarray):
    xf = np.ascontiguousarray(np.asarray(x, dtype=np.float32)).reshape(ROWS, SIZE)
    dg = np.ascontiguousarray(np.asarray(diag, dtype=np.float32))
    in_maps = []
    for c in range(N_CORES):
        pair, is_odd = divmod(c, 2)
        base = pair * PAIR_ROWS
        if is_odd:
            shard = xf[base + ROWS_EVEN : base + PAIR_ROWS]
        else:
            shard = np.zeros((ROWS_ODD, SIZE), dtype=np.float32)
            shard[:ROWS_EVEN] = xf[base : base + ROWS_EVEN]
        in_maps.append(
            {
                "x": shard.reshape(NT, P, SIZE),
                "diag": dg,
                "flag": np.array([[is_odd]], dtype=np.uint32),
            }
        )
    return in_maps


def _assemble(results) -> np.ndarray:
    out = np.empty((ROWS, SIZE), dtype=np.float32)
    for c in range(N_CORES):
        pair, is_odd = divmod(c, 2)
        base = pair * PAIR_ROWS
        yc = results[c]["y"].reshape(ROWS_ODD, SIZE)
        if is_odd:
            out[base + ROWS_EVEN : base + PAIR_ROWS] = yc
        else:
            out[base : base + ROWS_EVEN] = yc[:ROWS_EVEN]
    return out.reshape(BATCH, SEQ, SIZE)


def kernel(x: np.ndarray, diag: np.ndarray) -> np.ndarray:
    nc = _get_nc()
    res = run_bass_kernel_spmd(nc, _make_in_maps(x, diag), list(range(N_CORES)))
    return _assemble(res.results)


# revision 7
# speedup vs baseline: 1.0972x; 1.0972x over previous
"""Trainium2 Bass kernel for DiagonalLinear: y = x * diag (elementwise scale of last axis).

x: [4, 4096, 4096] f32, diag: [4096] f32 -> y: [4, 4096, 4096] f32.

Data-parallel over 8 NeuronCores: the 16384 rows (batch*seq) are split into
8 equal shards of 2048 rows; diag is replicated. Each core streams its
32 MiB shard through SBUF in 8 tiles of [128 partitions x 8192 floats]
(2 consecutive rows per partition), multiplies by a diag tile replicated
across partitions (diag is loaded once, 16 KiB, and broadcast on-chip),
and streams the result back out. Memory-bound: ~64 MiB of HBM traffic per
core, roofline ~160-190 us depending on HBM-stack arbitration.
"""

import numpy as np

import concourse.bass as bass
import concourse.bacc as bacc
import concourse.mybir as mybir
import concourse.tile as tile
from concourse.bass_utils import run_bass_kernel_spmd

BATCH, SEQ, SIZE = 4, 4096, 4096
N_CORES = 8
ROWS = BATCH * SEQ                   # 16384
ROWS_PER_CORE = ROWS // N_CORES      # 2048
P = 128                              # SBUF partitions
F = 8192                             # free-dim elements per partition per tile
ROWS_PER_PART = F // SIZE            # 2 consecutive rows per partition
T = ROWS_PER_CORE * SIZE // (P * F)  # 8 tiles of 4 MiB per core
FP32 = mybir.dt.float32

_built = None


def _build():
    """Build + schedule the per-core Tile kernel (same program on all 8 cores)."""
    nc = bacc.Bacc("TRN2", target_bir_lowering=False, debug=False)

    x = nc.dram_tensor("x", [T, P, F], FP32, kind="ExternalInput").ap()
    d = nc.dram_tensor("diag", [SIZE], FP32, kind="ExternalInput").ap()
    y = nc.dram_tensor("y", [T, P, F], FP32, kind="ExternalOutput").ap()

    with tile.TileContext(nc) as tc:
        with (
            tc.tile_pool(name="dpool", bufs=1) as dpool,
            tc.tile_pool(name="xpool", bufs=4) as xpool,
        ):
            # Load diag once (16 KiB) into partition 0, replicate across all
            # 128 partitions on-chip — no bulk HBM traffic for the broadcast.
            d0 = dpool.tile([1, SIZE], FP32)
            nc.sync.dma_start(out=d0[:], in_=d[None, :])
            dtile = dpool.tile([P, SIZE], FP32)
            nc.gpsimd.partition_broadcast(dtile[:], d0[:])

            for t in range(T):
                xt = xpool.tile([P, F], FP32)
                nc.sync.dma_start(out=xt[:], in_=x[t])
                for j in range(ROWS_PER_PART):
                    sl = xt[:, j * SIZE : (j + 1) * SIZE]
                    nc.vector.tensor_mul(sl, sl, dtile[:])
                nc.scalar.dma_start(out=y[t], in_=xt[:])

    nc.compile()
    return nc


def _get_nc():
    global _built
    if _built is None:
        _built = _build()
    return _built


def _make_in_maps(x: np.ndarray, diag: np.ndarray):
    xs = np.ascontiguousarray(np.asarray(x, dtype=np.float32)).reshape(
        N_CORES, T, P, F
    )
    dg = np.ascontiguousarray(np.asarray(diag, dtype=np.float32))
    return [{"x": xs[i], "diag": dg} for i in range(N_CORES)]


def _assemble(results) -> np.ndarray:
    out = np.stack([results[i]["y"] for i in range(N_CORES)])
    return out.reshape(BATCH, SEQ, SIZE)


def kernel(x: np.ndarray, diag: np.ndarray) -> np.ndarray:
    nc = _get_nc()
    res = run_bass_kernel_spmd(nc, _make_in_maps(x, diag), list(range(N_CORES)))
    return _assemble(res.results)
